# revision 48
# baseline (speedup 1.0000x reference)
"""BeansBackboneV2 sparse-attention block on 8 TRN2 NeuronCores.

Sharding: data-parallel over batch B=2 (4 cores per batch group); within a
group, TOKENS are sharded 256 per core (plus a replicated CLS column and a
dummy pad column so f32r matmuls keep an even moving dim).  Each core runs
all 16 heads for its token quarter, so the only collective is a 1MB->4MB
AllGather of the router k-projection feature chunks; proj/MLP are fully
local (full weights per core) and host assembly is pure concatenation.

The router is computed in exact fp32 (top-32 selection sits on near-ties,
so any rounding flips routes): LN1 stats, rq/rk projections, l2norm and
scores stay fp32 and bit-identical across cores.  Everything downstream of
route selection (QKV, attention, proj, LN2, MLP) runs the PE in fp32r
(1 cycle/row vs fp32's 4) with f32r-tagged producers.

Per core: q_r for its 256 patches, k_r for its 256 features of all patches
(gathered to full), scores/top-32/route-bias for its 256 rows only; the
bias matrix [key, query-quarter] feeds dense masked attention (bias 0 for
the CLS column, -87 for non-routed pairs); CLS's dense attention over all
S keys reuses the same loop via a key-0 accumulation step with a -87 bias
row for patch queries.

kernel(**inputs) takes the full unsharded inputs from setup_inputs() and
returns the full [2, 1025, 1024] output.
"""

import numpy as np

B, S, D, H, P = 2, 1025, 1024, 16, 1024
HD = D // H               # 64
TEMP = 0.1
SCALE = HD ** -0.5
EPS = 1e-5
EXCL = -87.0              # additive bias for non-routed pairs (exp -> ~1e-38)
NK = D // 128             # 8 contraction chunks
QT = P // 4               # token/feature quarter per core
QW = QT + 2               # quarter + CLS + dummy pad (even width for f32r)
SBLK = [(0, 512), (512, 512), (1024, 1)]          # token blocks of S=1025
VOFF = {
    'norm1_w': 0, 'norm1_b': 8, 'rq_b': 16, 'rk_b': 24,   # rk_b: 2 cols
    'proj_b': 32, 'norm2_w': 40, 'norm2_b': 48, 'fc2_b': 56,
    'qkv_bq': 64, 'qkv_bk': 72, 'qkv_bv': 80, 'fc1_b': 88,  # fc1_b: 32 cols
}
NV = 120

_CACHE = {}


def build_nc(sim_gelu=False, reps=1, no_cc=False, phases=99,
             fr_ln1=False, fr_ln2=True, fr_qkv=True, fr_attn=True,
             fr_proj=True):
    import concourse.bass as bass
    import concourse.bacc as bacc
    import concourse.mybir as mybir
    import concourse.tile as tile
    from concourse.masks import make_identity
    from contextlib import ExitStack

    f32 = mybir.dt.float32
    A = mybir.AluOpType
    AF = mybir.ActivationFunctionType
    X = mybir.AxisListType.X

    nc = bacc.Bacc("TRN2", target_bir_lowering=False, debug=False,
                   num_devices=8)
    f32r = mybir.dt.float32r
    bf16 = mybir.dt.bfloat16

    def mm(out, lhsT, rhs, **kw):
        if rhs.free_size() % 2:
            return nc.tensor.matmul(out, lhsT, rhs, **kw)
        return nc.tensor.matmul(out, lhsT.bitcast(f32r), rhs.bitcast(f32r), **kw)

    def frb(ap, flag):
        return ap.bitcast(f32r) if flag else ap

    def din(name, shape, dt=None):
        return nc.declare_dram_parameter(name, list(shape), dt or f32,
                                         isOutput=False)

    x_t = din("x_t", [D, S])
    xq_t = din("xq_t", [D, QW])
    rq_wT = din("rq_wT", [D, D])
    rkq_wT = din("rkq_wT", [D, QT])
    pos_bias_q = din("pos_bias_q", [QT, P])
    wqT = din("wqT", [D, D], bf16)
    wkT = din("wkT", [D, D], bf16)
    wvT = din("wvT", [D, D], bf16)
    projT = din("projT", [D, D], bf16)
    fc1T = din("fc1T", [D, 4 * D], bf16)
    fc2T = din("fc2T", [4 * D, D], bf16)
    vecs = din("vecs", [128, NV])
    y_t = nc.declare_dram_parameter("y_t", [D, QW], f32, isOutput=True)

    with tile.TileContext(nc) as tc:
      for _rep in range(reps):
        with ExitStack() as top:
                const = top.enter_context(tc.tile_pool(name="const", bufs=1))
                ones_raw = const.tile([128, 128], f32, tag="ones_raw", name="ones_raw")
                nc.vector.memset(ones_raw, 1.0)
                ones = const.tile([128, 128], f32, tag="ones", name="ones")
                nc.vector.tensor_copy(ones.bitcast(f32r), ones_raw)
                onesb = const.tile([128, 128], bf16, tag="onesb", name="onesb")
                nc.vector.memset(onesb, 1.0)
                ident = const.tile([128, 128], f32, tag="ident", name="ident")
                make_identity(nc, ident)
                vt = const.tile([128, NV], f32, tag="vt", name="vt")
                nc.sync.dma_start(vt, vecs[:, :])
                # key-0 bias row: EXCL for patch/dummy queries, 0 for CLS
                b0 = const.tile([1, QW], f32, tag="b0", name="b0")
                nc.vector.memset(b0, EXCL)
                nc.vector.memset(b0[:, QT:QT + 1], 0.0)

                def vcol(key, m):
                    return vt[:, VOFF[key] + m:VOFF[key] + m + 1]

                # scaled q bias: qkv_bq * SCALE (8 cols)
                sv = const.tile([128, 8], f32, tag="sv", name="sv")
                nc.vector.tensor_scalar_mul(
                    sv, vt[:, VOFF['qkv_bq']:VOFF['qkv_bq'] + 8], SCALE)

                stat = top.enter_context(tc.tile_pool(name="stat", bufs=1))
                scr = top.enter_context(tc.tile_pool(name="scr", bufs=2))

                # ---------------- helpers ----------------
                def layer_norm_T(src, dst_pool, wkey, bkey, tagp, out_f32r=False,
                                 stats_f32r=False, blocks=None, width=None,
                                 inplace=False, out_dt=None):
                    """src: 8 x [128,W] transposed-activation tiles -> normed."""
                    if blocks is None:
                        blocks, width = SBLK, S
                    with tc.tile_pool(name=f"lnp_{tagp}", bufs=2, space="PSUM") as lpp:
                        mean_b = stat.tile([128, width], f32, tag=f"mean_{tagp}",
                                           name=f"mean_{tagp}")
                        rstd_b = stat.tile([128, width], f32, tag=f"rstd_{tagp}",
                                           name=f"rstd_{tagp}")
                        for (soff, slen) in blocks:
                            ps_s = lpp.tile([128, 512], f32, tag="ln_s", name="ps_s")
                            ps_q = lpp.tile([128, 512], f32, tag="ln_q", name="ps_q")
                            mx = mm if stats_f32r else nc.tensor.matmul
                            on = ones if stats_f32r else ones_raw
                            for c in range(NK):
                                sq = scr.tile([128, 512], f32, tag="sq", name="sq")
                                sqw = sq[:, :slen].bitcast(f32r) if stats_f32r \
                                    else sq[:, :slen]
                                nc.scalar.activation(sqw,
                                                     src[c][:, soff:soff + slen], AF.Square)
                                mx(ps_s[:, :slen], on, src[c][:, soff:soff + slen],
                                   start=(c == 0), stop=(c == NK - 1))
                                mx(ps_q[:, :slen], on, sq[:, :slen],
                                   start=(c == 0), stop=(c == NK - 1))
                            m = mean_b[:, soff:soff + slen]
                            r = rstd_b[:, soff:soff + slen]
                            nc.vector.tensor_scalar_mul(m, ps_s[:, :slen], 1.0 / D)
                            nc.vector.tensor_scalar_mul(r, ps_q[:, :slen], 1.0 / D)  # E[x^2]
                            msq = scr.tile([128, 512], f32, tag="rs", name="msq")
                            nc.vector.tensor_mul(msq[:, :slen], m, m)
                            nc.vector.tensor_sub(r, r, msq[:, :slen])                # var
                            nc.vector.tensor_scalar_add(r, r, EPS)
                            nc.scalar.activation(r, r, AF.Sqrt)
                            nc.vector.reciprocal(r, r)
                        dst = []
                        for c in range(NK):
                            if inplace:
                                d = src[c]
                            else:
                                d = dst_pool.tile([128, width], out_dt or f32,
                                                  tag=f"{tagp}{c}",
                                                  name=f"{tagp}{c}")
                            dw = d.bitcast(f32r) if out_f32r else d
                            nc.vector.tensor_sub(dw, src[c], mean_b)
                            nc.vector.tensor_mul(dw, d, rstd_b)
                            nc.vector.tensor_scalar(dw, d, vcol(wkey, c), vcol(bkey, c),
                                                    A.mult, A.add)
                            dst.append(d)
                        return dst

                def gemm_T(wT_dram, Mo, act, act_off, Sw, evict, wtag, wsplit=None,
                           mode='f32r'):
                    """evict(m, soff, slen, ps) receives psum with
                    (wT.T @ act[:, act_off+soff : ...])[m*128:(m+1)*128]."""
                    nch = len(act)
                    if wsplit is None:
                        wsplit = 512 if Mo > 512 else Mo
                    wdt = bf16 if mode == 'bf16' else f32
                    with tc.tile_pool(name=f"wp_{wtag}", bufs=1) as wp, \
                         tc.tile_pool(name=f"gp_{wtag}", bufs=3, space="PSUM") as gpp:
                        for mg in range(Mo // wsplit):
                            wts = []
                            for c in range(nch):
                                w = wp.tile([128, wsplit], wdt, tag=f"{wtag}{c}",
                                            name=f"{wtag}{c}_{mg}")
                                wsrc = wT_dram[c * 128:(c + 1) * 128,
                                               mg * wsplit:(mg + 1) * wsplit]
                                if mode == 'f32r':
                                    nc.sync.dma_start(w.bitcast(f32r),
                                                      wsrc.bitcast(f32r))
                                else:
                                    nc.sync.dma_start(w, wsrc)
                                wts.append(w)
                            for ml in range(wsplit // 128):
                                m = mg * (wsplit // 128) + ml
                                for (soff, slen) in SBLK:
                                    if soff >= Sw:
                                        continue
                                    slen = min(slen, Sw - soff)
                                    ps = gpp.tile([128, 512], f32, tag="gp", name="ps")
                                    mmx = mm if mode == 'f32r' else nc.tensor.matmul
                                    for c in range(nch):
                                        mmx(
                                            ps[:, :slen], wts[c][:, ml * 128:(ml + 1) * 128],
                                            act[c][:, act_off + soff:act_off + soff + slen],
                                            start=(c == 0), stop=(c == nch - 1))
                                    evict(m, soff, slen, ps)

                def l2norm_T(tiles, n_cols):
                    with tc.tile_pool(name="l2p", bufs=2, space="PSUM") as l2p:
                        rinv = stat.tile([128, n_cols], f32, tag=f"rinv{n_cols}",
                                         name=f"rinv{n_cols}")
                        for hoff in range(0, n_cols, 512):
                            hlen = min(512, n_cols - hoff)
                            hs = slice(hoff, hoff + hlen)
                            ps = l2p.tile([128, 512], f32, tag="l2", name="ps_l2")[:, :hlen]
                            for c in range(NK):
                                sq = scr.tile([128, 512], f32, tag="sq", name="sq2")[:, :hlen]
                                nc.scalar.activation(sq, tiles[c][:, hs], AF.Square)
                                nc.tensor.matmul(ps, ones_raw, sq,
                                                 start=(c == 0), stop=(c == NK - 1))
                            r = rinv[:, hs]
                            nc.scalar.activation(r, ps, AF.Sqrt)
                            nc.vector.tensor_scalar_max(r, r, 1e-12)
                            nc.vector.reciprocal(r, r)
                        for c in range(NK):
                            nc.vector.tensor_mul(tiles[c], tiles[c], rinv)

                # ------------- phase 1: loads + LN1 full (fp32) --------------
                xqp = top.enter_context(tc.tile_pool(name="xqp", bufs=1))
                xq = []
                for c in range(NK):
                    t = xqp.tile([128, QW], f32, tag=f"xq{c}", name=f"xq{c}")
                    nc.sync.dma_start(t, xq_t[c * 128:(c + 1) * 128, :])
                    xq.append(t)

                rdram = top.enter_context(tc.tile_pool(name="rdram", bufs=1,
                                                       space="DRAM"))
                kr_in = rdram.tile([QT, P], f32, tag="kr_in", name="kr_in")
                kr_out = rdram.tile([P, P], f32, tag="kr_out", name="kr_out")

                x2p = top.enter_context(tc.tile_pool(name="x2p", bufs=1))
                ao_es = ExitStack()
                ao_pool = ao_es.enter_context(tc.tile_pool(name="ao_pool", bufs=1))
                bias_es = ExitStack()
                bias_pool = bias_es.enter_context(tc.tile_pool(name="bias_pool",
                                                               bufs=1))
                qkv_es = ExitStack()
                qkvp = qkv_es.enter_context(tc.tile_pool(name="qkvp", bufs=1))
                xnr_es = ExitStack()
                xnp2 = xnr_es.enter_context(tc.tile_pool(name="xnp2", bufs=1))

                def _close_stacks():
                    for _s in (xnr_es, qkv_es, bias_es, ao_es):
                        _s.close()

                def ev_r(dst, bk):
                    def ev(m, soff, slen, ps):
                        nc.scalar.activation(
                            dst[m][:, soff:soff + slen],
                            ps[:, :slen], AF.Identity, bias=vcol(bk, m))
                    return ev

                with tc.tile_pool(name="xnp", bufs=1) as xnp:
                    xT = []
                    for c in range(NK):
                        t = xnp.tile([128, S], f32, tag=f"xT{c}", name=f"xT{c}")
                        nc.sync.dma_start(t, x_t[c * 128:(c + 1) * 128, :])
                        xT.append(t)
                    xnT = layer_norm_T(xT, xnp, 'norm1_w', 'norm1_b',
                                       'xnT', stats_f32r=fr_ln1, inplace=True)

                    # local k_r feature chunk (256 rows of k_r^T) over all
                    # patches (fp32); the gather runs while other work proceeds
                    with tc.tile_pool(name="krcp", bufs=1) as krcp:
                        krc = [krcp.tile([128, P], f32, tag=f"krc{c}",
                                         name=f"krc{c}") for c in range(2)]
                        gemm_T(rkq_wT, QT, xnT, 1, P, ev_r(krc, 'rk_b'), "wrk",
                               wsplit=QT, mode='f32')
                        for c in range(2):
                            nc.sync.dma_start(kr_in[c * 128:(c + 1) * 128, :],
                                              krc[c])
                    if no_cc:
                        for g in range(4):
                            nc.sync.dma_start(kr_out[g * QT:(g + 1) * QT, :], kr_in)
                    else:
                        nc.gpsimd.collective_compute(
                            "AllGather", A.bypass,
                            replica_groups=[[0, 1, 2, 3], [4, 5, 6, 7]],
                            ins=[kr_in.opt()], outs=[kr_out.opt()])

                    # bf16 copies for the bf16 K/V gemms
                    xnR = []
                    for c in range(NK):
                        t = xnp2.tile([128, S], bf16, tag=f"xnR{c}", name=f"xnR{c}")
                        nc.scalar.copy(t, xnT[c])
                        xnR.append(t)

                # ------------- phase 2: LN1 quarter + q_r (fp32) + QKV -------
                biasT = [bias_pool.tile([128, QW], f32, tag=f"bT{c}", name=f"bT{c}")
                         for c in range(NK)]
                with ExitStack() as ph23:
                    qrp = ph23.enter_context(tc.tile_pool(name="qrp", bufs=1))
                    q_rT = [qrp.tile([128, QT], f32, tag=f"qr{c}", name=f"qr{c}")
                            for c in range(NK)]
                    with tc.tile_pool(name="xnqp", bufs=1) as xnqp:
                        xnq = layer_norm_T(xq, xnqp, 'norm1_w', 'norm1_b', 'xnq',
                                           blocks=[(0, QW)], width=QW)
                        gemm_T(rq_wT, D, xnq, 0, QT, ev_r(q_rT, 'rq_b'), "wrq",
                               wsplit=256, mode='f32')
                        # bf16 copy of the quarter for the bf16 Q gemm
                        xnqR = []
                        for c in range(NK):
                            t = xnqp.tile([128, QW], bf16, tag=f"xnqR{c}",
                                          name=f"xnqR{c}")
                            nc.scalar.copy(t, xnq[c])
                            xnqR.append(t)

                        # ---- Q/K projections (f32r) — overlap the gather ----
                        QTs = [qkvp.tile([128, QW], bf16, tag=f"QT{i}",
                                         name=f"QT{i}") for i in range(NK)]
                        KTt = [qkvp.tile([128, S], bf16, tag=f"KT{i}",
                                         name=f"KT{i}") for i in range(NK)]

                        def ev_q(m, soff, slen, ps):
                            nc.scalar.activation(
                                QTs[m][:, soff:soff + slen],
                                ps[:, :slen], AF.Identity,
                                bias=sv[:, m:m + 1], scale=SCALE)

                        def ev_k(m, soff, slen, ps):
                            nc.scalar.activation(
                                KTt[m][:, soff:soff + slen],
                                ps[:, :slen], AF.Identity, bias=vcol('qkv_bk', m))
                        gemm_T(wqT, D, xnqR, 0, QW, ev_q, "wq", mode='bf16')
                        gemm_T(wkT, D, xnR, 0, S, ev_k, "wk", mode='bf16')
                    l2norm_T(q_rT, QT)

                    krp = ph23.enter_context(tc.tile_pool(name="krp", bufs=1))
                    k_rT = [krp.tile([128, P], f32, tag=f"kr{c}", name=f"kr{c}")
                            for c in range(NK)]
                    for c in range(NK):
                        nc.sync.dma_start(k_rT[c], kr_out[c * 128:(c + 1) * 128, :])
                    l2norm_T(k_rT, P)

                    # --- phase 3: scores/top-32/bias for the local q-rows ---
                    with tc.tile_pool(name="bp", bufs=1) as bp, \
                         tc.tile_pool(name="scp", bufs=2, space="PSUM") as scp, \
                         tc.tile_pool(name="tp", bufs=2, space="PSUM") as tp:
                        for qb in range(QT // 128):
                            pb = bp.tile([128, P], f32, tag="pbt2", name="pb")
                            nc.sync.dma_start(pb, pos_bias_q[qb * 128:(qb + 1) * 128, :])
                            nc.vector.tensor_scalar_mul(pb, pb, 1.0 / TEMP)
                            tnat = bp.tile([128, P], f32, tag="tnat", name="tnat")
                            for nb in range(2):
                                ns = slice(nb * 512, nb * 512 + 512)
                                ps = scp.tile([128, 512], f32, tag="sc", name="ps_sc")
                                for c in range(NK):
                                    nc.tensor.matmul(
                                        ps, q_rT[c][:, qb * 128:(qb + 1) * 128],
                                        k_rT[c][:, ns],
                                        start=(c == 0), stop=(c == NK - 1))
                                nc.vector.scalar_tensor_tensor(tnat[:, ns], ps, 1.0 / TEMP,
                                                               pb[:, ns], A.mult, A.add)
                            # diagonal mask is baked into pos_bias_q host-side
                            # top-32 via 4 rounds of max8 + match_replace
                            t2 = bp.tile([128, P], f32, tag="pbt2", name="t2")
                            vals = bp.tile([128, 32], f32, tag="vals", name="vals")
                            src_mr = tnat
                            for r in range(4):
                                nc.vector.max(vals[:, r * 8:(r + 1) * 8], src_mr)
                                nc.vector.match_replace(t2, vals[:, r * 8:(r + 1) * 8],
                                                        src_mr, -1e30)
                                src_mr = t2
                            e32 = bp.tile([128, 32], f32, tag="e32", name="e32")
                            nc.scalar.activation(e32, vals, AF.Exp)
                            lse = bp.tile([128, 1], f32, tag="lse", name="lse")
                            nc.vector.tensor_reduce(lse, e32, X, A.add)
                            nc.scalar.activation(lse, lse, AF.Ln)
                            # bias = sel*(max(t-lse,-10) - EXCL) + EXCL, in place
                            bn = tnat
                            nc.vector.tensor_scalar(bn, tnat, lse[:, 0:1], -10.0,
                                                    A.subtract, A.max)
                            nc.vector.tensor_scalar_add(bn, bn, -EXCL)
                            nc.vector.scalar_tensor_tensor(bn, t2, -1e20, bn,
                                                           A.is_lt, A.mult)
                            nc.vector.tensor_scalar_add(bn, bn, EXCL)
                            for kb in range(8):
                                pt = tp.tile([128, 128], f32, tag="pt", name="pt")
                                nc.tensor.transpose(pt, bn[:, kb * 128:(kb + 1) * 128], ident)
                                nc.scalar.copy(biasT[kb][:, qb * 128:(qb + 1) * 128], pt)
                    for kb in range(8):
                        nc.vector.memset(biasT[kb][:, QT:QT + 1], 0.0)      # CLS
                        nc.vector.memset(biasT[kb][:, QT + 1:QT + 2], EXCL)  # pad

                if phases <= 3:
                    _close_stacks()
                    continue
                # ---------------- phase 4: V (f32r) --------------------------
                Vn = [qkvp.tile([128, D], bf16, tag=f"Vn{i}", name=f"Vn{i}")
                      for i in range(9)]
                bv_row = qkvp.tile([1, D], bf16, tag="bv_row", name="bv_row")
                # qkv_bv as a [1, 1024] row (PE transpose of 8 cols + 8 DMAs)
                with tc.tile_pool(name="vbp", bufs=1) as vbp, \
                     tc.tile_pool(name="vbps", bufs=1, space="PSUM") as vbps:
                    ptv = vbps.tile([128, 128], f32, tag="ptv", name="ptv")
                    nc.tensor.transpose(ptv[0:8, :], vt[:, VOFF['qkv_bv']:VOFF['qkv_bv'] + 8],
                                        ident)
                    s2 = vbp.tile([8, 128], bf16, tag="s2", name="s2")
                    nc.scalar.copy(s2, ptv[0:8, :])
                    for i in range(8):
                        nc.sync.dma_start(bv_row[0:1, i * 128:(i + 1) * 128],
                                          s2[i:i + 1, :])

                with tc.tile_pool(name="wvp", bufs=1) as wvp, \
                     tc.tile_pool(name="vps", bufs=2, space="PSUM") as vpsp:
                    vblocks = [(0, 1)] + [(1 + 128 * k, 128) for k in range(8)]
                    for half in range(2):
                        hsl = slice(half * 512, half * 512 + 512)
                        wvt = []
                        for c in range(NK):
                            w = wvp.tile([128, 512], bf16, tag=f"wv{c}",
                                         name=f"wv{c}_{half}")
                            nc.sync.dma_start(w, wvT[c * 128:(c + 1) * 128, hsl])
                            wvt.append(w)
                        for vi, (voff, vlen) in enumerate(vblocks):
                            ps = vpsp.tile([128, 512], f32, tag="vps", name="ps_v")
                            for c in range(NK):
                                nc.tensor.matmul(ps[:vlen, :],
                                                 xnR[c][:, voff:voff + vlen],
                                                 wvt[c], start=(c == 0), stop=False)
                            nc.tensor.matmul(ps[:vlen, :], onesb[0:1, 0:vlen],
                                             bv_row[0:1, hsl],
                                             start=False, stop=True)
                            nc.scalar.copy(Vn[vi][:vlen, hsl], ps[:vlen, :])
                xnr_es.close()

                if phases <= 4:
                    _close_stacks()
                    continue
                # ---------------- phase 5: attention (16 heads, f32r) --------
                aoutT = [ao_pool.tile([128, QW], bf16, tag=f"ao{i}", name=f"ao{i}")
                         for i in range(NK)]
                with tc.tile_pool(name="ep", bufs=4) as ep, \
                     tc.tile_pool(name="spp", bufs=3, space="PSUM") as spp, \
                     tc.tile_pool(name="pop", bufs=2, space="PSUM") as pop, \
                     tc.tile_pool(name="dnp", bufs=2, space="PSUM") as dnp:
                    for hl in range(H):
                        ti, ro = hl // 2, (hl % 2) * 64
                        rs = slice(ro, ro + 64)
                        hc = slice(64 * hl, 64 * hl + 64)
                        QTh = QTs[ti][rs, :]
                        KTh = KTt[ti][rs, :]
                        # key-0 (CLS key) scores for all QW queries
                        sp0 = spp.tile([128, 512], f32, tag="sp", name="sp0")
                        nc.tensor.matmul(sp0[0:1, :QW], KTh[:, 0:1], QTh,
                                         start=True, stop=True)
                        ek0 = ep.tile([1, QW], bf16, tag="ek0", name="ek0")
                        nc.vector.tensor_add(ek0, sp0[0:1, :QW], b0)
                        nc.scalar.activation(ek0, ek0, AF.Exp)
                        po = pop.tile([64, QW], f32, tag="po", name="po_")
                        dn = dnp.tile([64, QW], f32, tag="dn", name="dn_")
                        nc.tensor.matmul(po, Vn[0][0:1, hc], ek0,
                                         start=True, stop=False)
                        nc.tensor.matmul(dn, onesb[0:1, 0:64], ek0,
                                         start=True, stop=False)
                        for kb in range(8):
                            ks = slice(1 + 128 * kb, 1 + 128 * (kb + 1))
                            sp = spp.tile([128, 512], f32, tag="sp", name="sp_")
                            nc.tensor.matmul(sp[:, :QW], KTh[:, ks], QTh,
                                             start=True, stop=True)
                            ek = ep.tile([128, QW], bf16, tag="ek", name="ek")
                            nc.vector.tensor_add(ek, sp[:, :QW], biasT[kb])
                            nc.scalar.activation(ek, ek, AF.Exp)
                            nc.tensor.matmul(po, Vn[1 + kb][:, hc], ek,
                                             start=False, stop=(kb == 7))
                            nc.tensor.matmul(dn, onesb[:, 0:64], ek,
                                             start=False, stop=(kb == 7))
                        rec = ep.tile([64, QW], f32, tag="rec", name="rec")
                        nc.vector.reciprocal(rec, dn)
                        nc.vector.tensor_mul(aoutT[ti][rs, :], po, rec)

                qkv_es.close()
                bias_es.close()

                if phases <= 5:
                    ao_es.close()
                    continue
                # ------------- phase 6: proj + residual (f32r, local) --------
                x2T = []
                for c in range(NK):
                    x2T.append(x2p.tile([128, QW], f32, tag=f"x2T{c}",
                                        name=f"x2T{c}"))

                def ev_x2(m, soff, slen, ps):
                    t = scr.tile([128, 512], f32, tag="rs", name="prs")
                    nc.scalar.activation(t[:, :slen], ps[:, :slen], AF.Identity,
                                         bias=vcol('proj_b', m))
                    nc.vector.tensor_add(frb(x2T[m][:, soff:soff + slen], fr_ln2),
                                         t[:, :slen], xq[m][:, soff:soff + slen])
                gemm_T(projT, D, aoutT, 0, QW, ev_x2, "wp", mode='bf16')
                ao_es.close()

                # ---------------- phase 7/8: LN2 + full MLP (f32r) -----------
                with ExitStack() as ph8:
                    lp = ph8.enter_context(tc.tile_pool(name="lp", bufs=1))
                    ln2T = layer_norm_T(x2T, lp, 'norm2_w', 'norm2_b', 'l2T',
                                        stats_f32r=fr_ln2, out_dt=bf16,
                                        blocks=[(0, QW)], width=QW)
                    hT = [lp.tile([128, QW], bf16, tag=f"hT{c}", name=f"hT{c}")
                          for c in range(32)]

                    def ev_h(m, soff, slen, ps):
                        dst = hT[m][:, soff:soff + slen]
                        if not sim_gelu:
                            nc.scalar.activation(dst, ps[:, :slen], AF.Gelu,
                                                 bias=vcol('fc1_b', m))
                            return
                        # CoreSim has no Gelu LUT: tanh-approx (sim only)
                        nc.scalar.activation(dst, ps[:, :slen], AF.Identity,
                                             bias=vcol('fc1_b', m))
                        s1 = scr.tile([128, 512], f32, tag="gl1", name="s1")[:, :slen]
                        nc.scalar.activation(s1, dst, AF.Square)
                        nc.vector.tensor_scalar(s1, s1, 0.044715, 1.0, A.mult, A.add)
                        nc.vector.tensor_mul(s1, s1, dst)
                        nc.vector.tensor_scalar_mul(s1, s1, 0.7978845608028654)
                        nc.scalar.activation(s1, s1, AF.Tanh)
                        nc.vector.tensor_scalar(s1, s1, 1.0, 0.5, A.add, A.mult)
                        nc.vector.tensor_mul(dst, dst, s1)
                    gemm_T(fc1T, 4 * D, ln2T, 0, QW, ev_h, "w1", mode='bf16')

                    with tc.tile_pool(name="yp", bufs=2) as yp:
                        def ev_y(m, soff, slen, ps):
                            yt = yp.tile([128, QW], f32, tag="yt", name="yt")
                            t = scr.tile([128, 512], f32, tag="rs", name="yrs")
                            nc.scalar.activation(t[:, :slen], ps[:, :slen],
                                                 AF.Identity, bias=vcol('fc2_b', m))
                            nc.vector.tensor_add(yt[:, soff:soff + slen], t[:, :slen],
                                                 x2T[m][:, soff:soff + slen])
                            nc.sync.dma_start(y_t[m * 128:(m + 1) * 128, :], yt)
                        gemm_T(fc2T, D, hT, 0, QW, ev_y, "w2", mode='bf16')

    nc.compile()
    return nc


def _prep_in_maps(inputs):
    def c(a):
        return np.ascontiguousarray(np.asarray(a), dtype=np.float32)

    import ml_dtypes

    def cb(a):
        return np.ascontiguousarray(np.asarray(a)).astype(ml_dtypes.bfloat16)

    qkv_w = np.asarray(inputs['qkv_w'])
    qkv_b = np.asarray(inputs['qkv_b'])
    pos_bias_m = np.asarray(inputs['pos_bias']).copy()
    np.fill_diagonal(pos_bias_m, -1e9)   # bake the self-route mask
    wq_T = cb(qkv_w[0:D].T)
    wk_T = cb(qkv_w[D:2 * D].T)
    wv_T = cb(qkv_w[2 * D:].T)
    proj_T = cb(np.asarray(inputs['proj_w']).T)
    fc1_T = cb(np.asarray(inputs['fc1_w']).T)
    fc2_T = cb(np.asarray(inputs['fc2_w']).T)
    rq_T = c(np.asarray(inputs['rq_w']).T)
    in_maps = []
    for core in range(8):
        b, g = core // 4, core % 4
        qs = slice(QT * g, QT * (g + 1))
        v = np.zeros((128, NV), np.float32)
        for k in ('norm1_w', 'norm1_b', 'rq_b', 'proj_b',
                  'norm2_w', 'norm2_b', 'fc2_b'):
            arr = np.asarray(inputs[k])
            v[:, VOFF[k]:VOFF[k] + 8] = arr.reshape(8, 128).T
        v[:, VOFF['rk_b']:VOFF['rk_b'] + 2] = \
            np.asarray(inputs['rk_b'])[qs].reshape(2, 128).T
        v[:, VOFF['qkv_bq']:VOFF['qkv_bq'] + 8] = qkv_b[0:D].reshape(8, 128).T
        v[:, VOFF['qkv_bk']:VOFF['qkv_bk'] + 8] = qkv_b[D:2 * D].reshape(8, 128).T
        v[:, VOFF['qkv_bv']:VOFF['qkv_bv'] + 8] = qkv_b[2 * D:].reshape(8, 128).T
        v[:, VOFF['fc1_b']:VOFF['fc1_b'] + 32] = \
            np.asarray(inputs['fc1_b']).reshape(32, 128).T
        xb_t = c(np.asarray(inputs['x'])[b].T)
        in_maps.append({
            'x_t': xb_t,
            'xq_t': c(np.concatenate(
                [xb_t[:, 1 + QT * g:1 + QT * (g + 1)],
                 xb_t[:, 0:1], xb_t[:, 0:1]], axis=1)),
            'rq_wT': rq_T,
            'rkq_wT': c(np.asarray(inputs['rk_w'])[qs, :].T),
            'pos_bias_q': c(pos_bias_m[qs, :]),
            'wqT': wq_T,
            'wkT': wk_T,
            'wvT': wv_T,
            'projT': proj_T,
            'fc1T': fc1_T,
            'fc2T': fc2_T,
            'vecs': c(v),
        })
    return in_maps


def get_nc(sim_gelu=False, reps=1, no_cc=False, phases=99):
    import os
    fr = {}
    for k in ('ln1', 'ln2', 'qkv', 'attn', 'proj'):
        vv = os.environ.get(f'FR_{k.upper()}')
        if vv is not None:
            fr[f'fr_{k}'] = bool(int(vv))
    key = f'nc{sim_gelu}_{reps}_{no_cc}_{phases}_{sorted(fr.items())}'
    if key not in _CACHE:
        _CACHE[key] = build_nc(sim_gelu, reps, no_cc, phases, **fr)
    return _CACHE[key]


def assemble(results):
    out = np.zeros((B, S, D), np.float32)
    for b in range(2):
        out[b, 0] = results[4 * b]['y_t'][:, QT]
        for g in range(4):
            out[b, 1 + QT * g:1 + QT * (g + 1)] = \
                results[4 * b + g]['y_t'][:, 0:QT].T
    return out


def kernel(**inputs):
    from concourse.bass_utils import run_bass_kernel_spmd
    nc = get_nc()
    in_maps = _prep_in_maps(inputs)
    res = run_bass_kernel_spmd(nc, in_maps, list(range(8))).results
    return assemble(res)
